# revision 2
# baseline (speedup 1.0000x reference)
"""Trainium2 Bass kernel for the retrieval-KNN correlation problem (v3).

Problem (per batch element b):
    idx[k,p]   = x[b,k,p] + 64*y[b,k,p]              (pixel coords into ref map)
    S[k,p]     = sum_c ref[b,c,idx[k,p]] * inp[b,c,p]
    best[p]    = argmax_k S[k,p]        (first occurrence on ties)
    out_x[p]   = x[b,best[p],p],  out_y[p] = y[b,best[p],p]

Sharding: 8 cores = (batch b = core//2, pixel half = core%2); no cross-core
communication.

v3 replaces the Q7 ap_gather pipeline (26ns/index wall -> ~295us/core) with
SWDGE dma_gather: ref stays in HBM pixel-major (4096 x 256 fp32 rows); each
(candidate,pixel) pair is one 1KB descriptor gathered straight to SBUF by the
16 DMA engines (0.34ns/desc Q7 gen + ~2.84ns/desc aggregate transfer).
num_idxs per instruction is capped (HW-bisected: 512/1024 OK, 4096 crashes
the exec unit - descriptor-ring capacity), so the 32768 pairs/core stream
through CP-pair chunks, each covering a pixel-slice of ONE candidate:
dst[j%128, j//128, :] with pair j = k*2048 + px puts pixel 128m+p on
partition p, so the DVE multiply uses the pixel-major inp tile directly
(no replication, no broadcast) and chunk scores land contiguously in a
[128, 16k+m] score tile whose argmax-over-k is a flat halving tree.
fp32 end to end (16-bit features flip argmaxes; min top-2 gap 1.7e-5).
"""

import os
import numpy as np
from contextlib import ExitStack

import concourse.bacc as bacc
import concourse.bass as bass
import concourse.mybir as mybir
import concourse.tile as tile
from concourse import bass_utils

B, K, CN, H, W = 4, 16, 256, 64, 64
HW = H * W
HALF = HW // 2        # 2048 pixels per core
NCORES = 8
CP = int(os.environ.get("CP", "1024"))  # HW-validated chunk size   # pairs per gather chunk
assert HALF % CP == 0
NCH = K * HALF // CP  # chunks per core
PXC = CP              # pixels per chunk (one candidate per chunk)
KPC = HALF // CP      # chunks per candidate
SLOTS = CP // 128     # dst slots per partition per chunk
IW = CP // 16         # idx words per partition per chunk
NM = HALF // 128      # 16 pixel groups (slot m)
MPC = PXC // 128      # m-groups per chunk

f32 = mybir.dt.float32
i16 = mybir.dt.int16

GBUFS = max(3, min(8, (96 * 1024) // (SLOTS * CN * 4)))  # ring depth


def build_program():
    nc = bacc.Bacc("TRN2", target_bir_lowering=False, debug=False,
                   num_swdge_queues=4)

    ref_d = nc.dram_tensor("refrows", (HW, CN), f32, kind="ExternalInput")
    inp_d = nc.dram_tensor("inp", (128, NM * CN), f32, kind="ExternalInput")
    gi_d = nc.dram_tensor("gidx", (128, NCH * IW), i16, kind="ExternalInput")
    xt_d = nc.dram_tensor("xt", (128, K * NM), f32, kind="ExternalInput")
    yt_d = nc.dram_tensor("yt", (128, K * NM), f32, kind="ExternalInput")
    revc_d = nc.dram_tensor("revc", (128, K * NM), f32, kind="ExternalInput")
    ox_d = nc.dram_tensor("ox", (128, NM), f32, kind="ExternalOutput")
    oy_d = nc.dram_tensor("oy", (128, NM), f32, kind="ExternalOutput")

    with ExitStack() as ctx:
        tc = ctx.enter_context(tile.TileContext(nc))
        pers = ctx.enter_context(tc.tile_pool(name="pers", bufs=1))
        gpool = ctx.enter_context(tc.tile_pool(name="g", bufs=GBUFS))
        epool = ctx.enter_context(tc.tile_pool(name="e", bufs=1))

        gidx = pers.tile([128, NCH * IW], i16, tag="gidx")
        inp = pers.tile([128, NM * CN], f32, tag="inp")
        spp = pers.tile([128, K * NM], f32, tag="spp")
        xt = pers.tile([128, K * NM], f32, tag="xt")
        yt = pers.tile([128, K * NM], f32, tag="yt")
        revc = pers.tile([128, K * NM], f32, tag="revc")
        oxv = pers.tile([128, NM], f32, tag="oxv")
        oyv = pers.tile([128, NM], f32, tag="oyv")

        nc.sync.dma_start(gidx[:], gi_d.ap())

        for c in range(NCH):
            g = gpool.tile([128, SLOTS * CN], f32, tag="g", name=f"g{c}")
            # queue_num selects the Q7 cpu pair (ucode: cpu_id/2 == queue_num);
            # round-robin over all 4 queues lets desc-gen for up to 4 gathers
            # overlap on disjoint cpu pairs.
            nc.gpsimd.dma_gather(
                g[:].rearrange("p (i c) -> p i c", c=CN),
                ref_d.ap(),
                gidx[:, IW * c:IW * (c + 1)],
                CP, CP, CN,
                queue_num=c % 4,
            )
            if c == 0:
                # needed first by chunk 0's multiply / the epilogue; issued
                # after the first gather so they stay out of its wait set.
                nc.sync.dma_start(inp[:], inp_d.ap())
                nc.sync.dma_start(xt[:], xt_d.ap())
                nc.sync.dma_start(yt[:], yt_d.ap())
                nc.sync.dma_start(revc[:], revc_d.ap())
            # chunk c = candidate c//KPC, pixels [PXC*(c%KPC), +PXC)
            m0 = MPC * (c % KPC)
            gv = g[:].rearrange("p (i c) -> p i c", c=CN)
            iv = inp[:, CN * m0:CN * (m0 + MPC)].rearrange(
                "p (m c) -> p m c", c=CN)
            nc.vector.tensor_mul(gv, gv, iv)
            nc.vector.tensor_reduce(
                spp[:, SLOTS * c:SLOTS * (c + 1)], gv,
                axis=mybir.AxisListType.X, op=mybir.AluOpType.add)

        # ---- epilogue: per-pixel argmax over k (slot i = 16k + m) ---------
        # flat halving tree works because k is slot-major: k<8 <=> i<128.
        def tree(dst, src, op, width):
            cur = src
            w = 8 * width
            while w >= width:
                out = dst if w == width else epool.tile(
                    [128, w], f32, tag=f"tr{w}_{op}", name=f"tr{w}_{op}")
                nc.vector.tensor_tensor(
                    out[:, 0:w], cur[:, 0:w], cur[:, w:2 * w], op=op)
                cur = out
                w //= 2
            return dst

        gmax = epool.tile([128, NM], f32, tag="gmax")
        ohall = epool.tile([128, K * NM], f32, tag="ohall")
        t1 = epool.tile([128, K * NM], f32, tag="t1")
        r1 = epool.tile([128, NM], f32, tag="r1")
        oh1 = epool.tile([128, K * NM], f32, tag="oh1")
        sel = epool.tile([128, K * NM], f32, tag="sel")

        def grp(ap):
            return ap.rearrange("p (k m) -> p k m", m=NM)

        tree(gmax, spp[:], mybir.AluOpType.max, NM)
        gb = gmax[:].unsqueeze(1).broadcast_to((128, K, NM))
        nc.vector.tensor_tensor(grp(ohall[:]), grp(spp[:]), gb,
                                op=mybir.AluOpType.is_equal)
        nc.vector.tensor_mul(t1[:], ohall[:], revc[:])
        tree(r1, t1[:], mybir.AluOpType.max, NM)
        rb = r1[:].unsqueeze(1).broadcast_to((128, K, NM))
        nc.vector.tensor_tensor(grp(oh1[:]), grp(t1[:]), rb,
                                op=mybir.AluOpType.is_equal)
        nc.vector.tensor_mul(sel[:], oh1[:], xt[:])
        tree(oxv, sel[:], mybir.AluOpType.add, NM)
        nc.vector.tensor_mul(sel[:], oh1[:], yt[:])
        tree(oyv, sel[:], mybir.AluOpType.add, NM)

        nc.sync.dma_start(ox_d.ap(), oxv[:])
        nc.sync.dma_start(oy_d.ap(), oyv[:])

    nc.compile()
    return nc


def pixel_major(a):
    """(K, HALF) -> (128, K*NM) with out[p, NM*k + m] = a[k, 128*m + p]."""
    return np.ascontiguousarray(
        a.reshape(K, NM, 128).transpose(2, 0, 1).reshape(128, K * NM))


def make_in_maps(input_features, ref_features, aggregated_x, aggregated_y):
    revc = np.tile(
        np.repeat(np.arange(K, 0, -1, dtype=np.float32), NM).reshape(1, K * NM),
        (128, 1))
    in_maps = []
    ref_cache = {}
    for core in range(NCORES):
        b, h = core // 2, core % 2
        sl = slice(h * HALF, (h + 1) * HALF)
        if b not in ref_cache:
            ref_cache[b] = np.ascontiguousarray(
                ref_features[b].reshape(CN, HW).T)
        x = aggregated_x[b].reshape(K, HW)[:, sl]
        y = aggregated_y[b].reshape(K, HW)[:, sl]
        idx = (x + y * W).astype(np.int16)  # (K, HALF)
        # chunk c covers pairs j = CP*c + jj; wrap w[jj%16, jj//16].
        flat = idx.reshape(NCH, CP)
        wrapped = flat.reshape(NCH, IW, 16).transpose(0, 2, 1)  # (NCH,16,IW)
        gi = np.tile(
            wrapped.transpose(1, 0, 2).reshape(16, NCH * IW), (8, 1))
        ippp = input_features[b].reshape(CN, HW)[:, sl].T  # (HALF, CN)
        ippp = ippp.reshape(NM, 128, CN).transpose(1, 0, 2).reshape(128, NM * CN)
        in_maps.append({
            "refrows": ref_cache[b],
            "inp": np.ascontiguousarray(ippp),
            "gidx": np.ascontiguousarray(gi),
            "xt": pixel_major(x),
            "yt": pixel_major(y),
            "revc": revc,
        })
    return in_maps


def assemble_outputs(results):
    offset_x = np.empty((B, 1, H, W), dtype=np.float32)
    offset_y = np.empty((B, 1, H, W), dtype=np.float32)
    for core in range(NCORES):
        b, h = core // 2, core % 2
        sl = slice(h * HALF, (h + 1) * HALF)
        offset_x[b, 0].reshape(HW)[sl] = results[core]["ox"].T.reshape(HALF)
        offset_y[b, 0].reshape(HW)[sl] = results[core]["oy"].T.reshape(HALF)
    return offset_x, offset_y


_PROGRAM = None


def kernel(input_features, ref_features, aggregated_x, aggregated_y):
    global _PROGRAM
    if _PROGRAM is None:
        _PROGRAM = build_program()
    nc = _PROGRAM
    in_maps = make_in_maps(input_features, ref_features, aggregated_x, aggregated_y)
    res = bass_utils.run_bass_kernel_spmd(nc, in_maps, core_ids=list(range(NCORES)))
    return assemble_outputs(res.results)


# revision 5
# speedup vs baseline: 1.4333x; 1.4333x over previous
"""Trainium2 Bass kernel for the retrieval-KNN correlation problem.

Problem (per batch element b):
    idx[k,p]   = x[b,k,p] + 64*y[b,k,p]              (pixel coords into ref map)
    S[k,p]     = sum_c ref[b,c,idx[k,p]] * inp[b,c,p]
    best[p]    = argmax_k S[k,p]        (first occurrence on ties)
    out_x[p]   = x[b,best[p],p],  out_y[p] = y[b,best[p],p]

Sharding: 8 cores = (batch b = core//2, pixel half = core%2); no cross-core
communication.

Architecture (HW exec 185.7us vs the 492.8us ap_gather baseline, exact
match): the per-(candidate,pixel) gather of 256-channel fp32 rows moves off
the Q7 (ap_gather: 26ns/index -> ~295us/core) onto the 16 DMA engines via
SWDGE dma_gather - ref stays in HBM pixel-major (4096 x 256 fp32); each
pair is one 1KB descriptor. Two HW-bisected constraints shape the kernel:
num_idxs <= 1024 per instruction (1536+ overflows the per-engine descriptor
ring and kills the exec unit), and the Q7 descriptor GENERATION costs
~8.4ns/desc on the cpu pair selected by queue_num - so the 32768 pairs/core
stream through 32 chunks of 1024 round-robined over all 4 SWDGE queues
(num_swdge_queues=4), overlapping generation across cpu pairs (~2x).

Pair ordering j = k*2048 + px makes dst[j%128, j//128, :] put pixel 128m+p
on partition p with one candidate per chunk, so the DVE multiply uses the
pixel-major inp tile directly (no replication, no broadcast) and chunk
scores land contiguously in a [128, 16k+m] score tile whose argmax-over-k
is a flat halving tree (k<8 <=> slot<128) with the descending-weight
first-occurrence trick. An 8-deep gather ring keeps gen/transfer/DVE
streaming (shallow rings lockstep), and the serial head chunk is split
into two 512-descriptor gathers on queues 0/1 to start the DVE earlier.
fp32 end to end: 16-bit features flip argmaxes (min top-2 gap 1.7e-5,
absmax-graded). HW: 181627ns exact (engine busy: DVE ~148us and Q7 gen
~136us co-walled, DMA ~115us, PE/ACT idle).
"""

import numpy as np
from contextlib import ExitStack

import concourse.bacc as bacc
import concourse.bass as bass
import concourse.mybir as mybir
import concourse.tile as tile
from concourse import bass_utils

B, K, CN, H, W = 4, 16, 256, 64, 64
HW = H * W
HALF = HW // 2        # 2048 pixels per core
NCORES = 8
CP = 1024             # pairs per gather chunk (HW limit: <=1024 per SWDGE instr)
assert HALF % CP == 0
NCH = K * HALF // CP  # chunks per core
PXC = CP              # pixels per chunk (one candidate per chunk)
KPC = HALF // CP      # chunks per candidate
SLOTS = CP // 128     # dst slots per partition per chunk
IW = CP // 16         # idx words per partition per chunk
NM = HALF // 128      # 16 pixel groups (slot m)
MPC = PXC // 128      # m-groups per chunk

f32 = mybir.dt.float32
i16 = mybir.dt.int16

GBUFS = max(3, min(8, (96 * 1024) // (SLOTS * CN * 4)))  # ring depth


def build_program():
    nc = bacc.Bacc("TRN2", target_bir_lowering=False, debug=False,
                   num_swdge_queues=4)

    ref_d = nc.dram_tensor("refrows", (HW, CN), f32, kind="ExternalInput")
    inp_d = nc.dram_tensor("inp", (128, NM * CN), f32, kind="ExternalInput")
    gi_d = nc.dram_tensor("gidx", (128, NCH * IW), i16, kind="ExternalInput")
    xt_d = nc.dram_tensor("xt", (128, K * NM), f32, kind="ExternalInput")
    yt_d = nc.dram_tensor("yt", (128, K * NM), f32, kind="ExternalInput")
    revc_d = nc.dram_tensor("revc", (128, K * NM), f32, kind="ExternalInput")
    ox_d = nc.dram_tensor("ox", (128, NM), f32, kind="ExternalOutput")
    oy_d = nc.dram_tensor("oy", (128, NM), f32, kind="ExternalOutput")

    with ExitStack() as ctx:
        tc = ctx.enter_context(tile.TileContext(nc))
        pers = ctx.enter_context(tc.tile_pool(name="pers", bufs=1))
        gpool = ctx.enter_context(tc.tile_pool(name="g", bufs=GBUFS))
        epool = ctx.enter_context(tc.tile_pool(name="e", bufs=1))

        gidx = pers.tile([128, NCH * IW], i16, tag="gidx")
        inp = pers.tile([128, NM * CN], f32, tag="inp")
        spp = pers.tile([128, K * NM], f32, tag="spp")
        xt = pers.tile([128, K * NM], f32, tag="xt")
        yt = pers.tile([128, K * NM], f32, tag="yt")
        revc = pers.tile([128, K * NM], f32, tag="revc")
        oxv = pers.tile([128, NM], f32, tag="oxv")
        oyv = pers.tile([128, NM], f32, tag="oyv")

        # chunk 0 is split into two 512-pair gathers on queues 0/1: they
        # are the serial head of the pipeline (nothing to overlap with yet),
        # so halving the first gen + transfer starts the DVE ~4us earlier.
        # SPECS entries: (word offset into gidx, m-group base, n m-groups, q)
        SPECS = [(0, 0, MPC // 2, 0), (IW // 2, MPC // 2, MPC // 2, 1)]
        for c in range(1, NCH):
            SPECS.append((IW * c, MPC * (c % KPC), MPC, (c + 1) % 4))
        # chunk-0's idx words land first so its gather isn't gated on the
        # full index load.
        nc.sync.dma_start(gidx[:, 0:IW], gi_d.ap()[:, 0:IW])
        nc.sync.dma_start(gidx[:, IW:], gi_d.ap()[:, IW:])

        for si, (w0, m0, nm, qn) in enumerate(SPECS):
            np_ = 128 * nm
            g = gpool.tile([128, MPC * CN], f32, tag="g", name=f"g{si}")
            nc.gpsimd.dma_gather(
                g[:, 0:nm * CN].rearrange("p (i c) -> p i c", c=CN),
                ref_d.ap(),
                gidx[:, w0:w0 + np_ // 16],
                np_, np_, CN,
                queue_num=qn,
            )
            if si == 0:
                # needed first by chunk 0's multiply / the epilogue; issued
                # after the first gather so they stay out of its wait set
                # (and BEFORE the first tensor_mul that reads inp).
                nc.sync.dma_start(inp[:], inp_d.ap())
                nc.sync.dma_start(xt[:], xt_d.ap())
                nc.sync.dma_start(yt[:], yt_d.ap())
                nc.sync.dma_start(revc[:], revc_d.ap())
            gv = g[:, 0:nm * CN].rearrange("p (i c) -> p i c", c=CN)
            iv = inp[:, CN * m0:CN * (m0 + nm)].rearrange(
                "p (m c) -> p m c", c=CN)
            nc.vector.tensor_mul(gv, gv, iv)
            s0 = w0 // 8  # global slot base = (16 * w0) / 128
            nc.vector.tensor_reduce(
                spp[:, s0:s0 + nm], gv,
                axis=mybir.AxisListType.X, op=mybir.AluOpType.add)

        # ---- epilogue: per-pixel argmax over k (slot i = 16k + m) ---------
        # flat halving tree works because k is slot-major: k<8 <=> i<128.
        def tree(dst, src, op, width):
            cur = src
            w = 8 * width
            while w >= width:
                out = dst if w == width else epool.tile(
                    [128, w], f32, tag=f"tr{w}_{op}", name=f"tr{w}_{op}")
                nc.vector.tensor_tensor(
                    out[:, 0:w], cur[:, 0:w], cur[:, w:2 * w], op=op)
                cur = out
                w //= 2
            return dst

        gmax = epool.tile([128, NM], f32, tag="gmax")
        ohall = epool.tile([128, K * NM], f32, tag="ohall")
        t1 = epool.tile([128, K * NM], f32, tag="t1")
        r1 = epool.tile([128, NM], f32, tag="r1")
        oh1 = epool.tile([128, K * NM], f32, tag="oh1")
        sel = epool.tile([128, K * NM], f32, tag="sel")

        def grp(ap):
            return ap.rearrange("p (k m) -> p k m", m=NM)

        tree(gmax, spp[:], mybir.AluOpType.max, NM)
        gb = gmax[:].unsqueeze(1).broadcast_to((128, K, NM))
        nc.vector.tensor_tensor(grp(ohall[:]), grp(spp[:]), gb,
                                op=mybir.AluOpType.is_equal)
        nc.vector.tensor_mul(t1[:], ohall[:], revc[:])
        tree(r1, t1[:], mybir.AluOpType.max, NM)
        rb = r1[:].unsqueeze(1).broadcast_to((128, K, NM))
        nc.vector.tensor_tensor(grp(oh1[:]), grp(t1[:]), rb,
                                op=mybir.AluOpType.is_equal)
        nc.vector.tensor_mul(sel[:], oh1[:], xt[:])
        tree(oxv, sel[:], mybir.AluOpType.add, NM)
        nc.vector.tensor_mul(sel[:], oh1[:], yt[:])
        tree(oyv, sel[:], mybir.AluOpType.add, NM)

        nc.sync.dma_start(ox_d.ap(), oxv[:])
        nc.sync.dma_start(oy_d.ap(), oyv[:])

    nc.compile()
    return nc


def pixel_major(a):
    """(K, HALF) -> (128, K*NM) with out[p, NM*k + m] = a[k, 128*m + p]."""
    return np.ascontiguousarray(
        a.reshape(K, NM, 128).transpose(2, 0, 1).reshape(128, K * NM))


def make_in_maps(input_features, ref_features, aggregated_x, aggregated_y):
    revc = np.tile(
        np.repeat(np.arange(K, 0, -1, dtype=np.float32), NM).reshape(1, K * NM),
        (128, 1))
    in_maps = []
    ref_cache = {}
    for core in range(NCORES):
        b, h = core // 2, core % 2
        sl = slice(h * HALF, (h + 1) * HALF)
        if b not in ref_cache:
            ref_cache[b] = np.ascontiguousarray(
                ref_features[b].reshape(CN, HW).T)
        x = aggregated_x[b].reshape(K, HW)[:, sl]
        y = aggregated_y[b].reshape(K, HW)[:, sl]
        idx = (x + y * W).astype(np.int16)  # (K, HALF)
        # chunk c covers pairs j = CP*c + jj; wrap w[jj%16, jj//16].
        flat = idx.reshape(NCH, CP)
        wrapped = flat.reshape(NCH, IW, 16).transpose(0, 2, 1)  # (NCH,16,IW)
        gi = np.tile(
            wrapped.transpose(1, 0, 2).reshape(16, NCH * IW), (8, 1))
        ippp = input_features[b].reshape(CN, HW)[:, sl].T  # (HALF, CN)
        ippp = ippp.reshape(NM, 128, CN).transpose(1, 0, 2).reshape(128, NM * CN)
        in_maps.append({
            "refrows": ref_cache[b],
            "inp": np.ascontiguousarray(ippp),
            "gidx": np.ascontiguousarray(gi),
            "xt": pixel_major(x),
            "yt": pixel_major(y),
            "revc": revc,
        })
    return in_maps


def assemble_outputs(results):
    offset_x = np.empty((B, 1, H, W), dtype=np.float32)
    offset_y = np.empty((B, 1, H, W), dtype=np.float32)
    for core in range(NCORES):
        b, h = core // 2, core % 2
        sl = slice(h * HALF, (h + 1) * HALF)
        offset_x[b, 0].reshape(HW)[sl] = results[core]["ox"].T.reshape(HALF)
        offset_y[b, 0].reshape(HW)[sl] = results[core]["oy"].T.reshape(HALF)
    return offset_x, offset_y


_PROGRAM = None


def kernel(input_features, ref_features, aggregated_x, aggregated_y):
    global _PROGRAM
    if _PROGRAM is None:
        _PROGRAM = build_program()
    nc = _PROGRAM
    in_maps = make_in_maps(input_features, ref_features, aggregated_x, aggregated_y)
    res = bass_utils.run_bass_kernel_spmd(nc, in_maps, core_ids=list(range(NCORES)))
    return assemble_outputs(res.results)
